# revision 3
# baseline (speedup 1.0000x reference)
"""V2 Trainium2 Bass kernel for BertSelfAttention (relative_key_query).

Data-parallel over batch (8 cores). Per-core design:
- host-side layout prep: hsT (transposed hidden states), reordered bf16
  weights resident in SBUF (loaded once)
- bf16 projections/tables/qT/kT/vh/cs, fp8 relative-position windows
- skewed SBUF->SBUF DMA for the diagonal extraction (no DRAM round trip)
- 512-wide PSUM pipeline (single shared 1-bank pool + 2-bank ctx pool)
- window copies split across ACT/DVE; batched per-head epilogue
"""

import os
import numpy as np

import concourse.bacc as bacc
import concourse.mybir as mybir
import concourse.tile as tile

f32 = mybir.dt.float32
f32r = mybir.dt.float32r
bf16 = mybir.dt.bfloat16
fp8 = mybir.dt.float8e4

S = 1024
D = 1024
H = 16
DH = 64
NT = 8
WIN = 1152
NPAIR = 8


def host_prep(hidden_states, attention_mask, W_qkv, b_qkv, dist_emb):
    import ml_dtypes
    f8 = ml_dtypes.float8_e4m3fn
    b16 = ml_dtypes.bfloat16

    B = hidden_states.shape[0]
    W = np.asarray(W_qkv, dtype=np.float32)
    b = np.asarray(b_qkv, dtype=np.float32)
    T = np.asarray(dist_emb, dtype=np.float32)

    # qk column reorder: pair t cols = [head 2t dims | head 2t+1 dims]
    qcols = np.zeros((8, 128), dtype=np.int64)
    kcols = np.zeros((8, 128), dtype=np.int64)
    for t in range(8):
        for j in range(128):
            h = 2 * t + (j >= 64)
            d = j % 64
            qcols[t, j] = h * 192 + d
            kcols[t, j] = h * 192 + 64 + d
    qk_idx = np.concatenate([qcols.reshape(-1), kcols.reshape(-1)])
    WQK = np.ascontiguousarray(W[:, qk_idx])               # (1024, 2048)
    # resident layout [128, 8, 2048]: [p, it, c] = WQK[128*it + p, c]
    WQKb = np.ascontiguousarray(
        WQK.reshape(8, 128, 2048).transpose(1, 0, 2)).astype(b16)
    bQK = np.ascontiguousarray(b[qk_idx].reshape(16, 128).T)  # (128, 16) f32

    vidx = np.array([h * 192 + 128 + d for h in range(H) for d in range(DH)])
    WV = np.ascontiguousarray(W[:, vidx])
    WVb = np.ascontiguousarray(
        WV.reshape(8, 128, 1024).transpose(1, 0, 2)).astype(b16)
    bV = np.ascontiguousarray(b[vidx].reshape(1, 1024))  # f32

    T2 = np.zeros((128, 2048), dtype=np.float32)
    T2[0:64, 0:2047] = T.T
    T2[64:128, 0:2047] = T.T
    T2R = np.zeros((128, 2048), dtype=np.float32)
    T2R[0:64, 0:2047] = T.T[:, ::-1]
    T2R[64:128, 0:2047] = T.T[:, ::-1]

    ones_r = np.ones((1, 128), dtype=np.float32)
    id8_h = np.eye(128, dtype=np.float32).astype(f8)
    idb_h = np.eye(65, dtype=np.float32).astype(b16)

    mask = np.asarray(attention_mask, dtype=np.float32).reshape(B, S)
    in_maps = []
    for bi in range(B):
        mhat = np.ascontiguousarray(mask[bi].reshape(8, 128).T)
        # hsT layout [128, 8, 1024]: [p, it, l] = hs[l, 128*it + p]
        hsT = np.ascontiguousarray(
            np.asarray(hidden_states[bi], dtype=np.float32)
            .T.reshape(8, 128, 1024).transpose(1, 0, 2)).astype(b16)
        in_maps.append({
            "hsT": hsT,
            "wqkb": WQKb, "bqk": bQK, "wvb": WVb, "bv": bV,
            "t2": T2.astype(b16), "t2r": T2R.astype(b16),
            "ones_r": ones_r, "mhat": mhat,
            "id8_h": id8_h, "idb_h": idb_h,
        })
    return in_maps


def build_program(npair=NPAIR):
    nc = bacc.Bacc()
    hsT_d = nc.declare_dram_parameter("hsT", [128, 8, 1024], bf16, isOutput=False)
    wqk_d = nc.declare_dram_parameter("wqkb", [128, 8, 2048], bf16, isOutput=False)
    bqk_d = nc.declare_dram_parameter("bqk", [128, 16], f32, isOutput=False)
    wv_d = nc.declare_dram_parameter("wvb", [128, 8, 1024], bf16, isOutput=False)
    bv_d = nc.declare_dram_parameter("bv", [1, 1024], f32r, isOutput=False)
    t2_d = nc.declare_dram_parameter("t2", [128, 2048], bf16, isOutput=False)
    t2r_d = nc.declare_dram_parameter("t2r", [128, 2048], bf16, isOutput=False)
    ones_d = nc.declare_dram_parameter("ones_r", [1, 128], f32r, isOutput=False)
    mhat_d = nc.declare_dram_parameter("mhat", [128, 8], f32, isOutput=False)
    id8_d = nc.declare_dram_parameter("id8_h", [128, 128], fp8, isOutput=False)
    idb_d = nc.declare_dram_parameter("idb_h", [65, 65], bf16, isOutput=False)
    out_d = nc.declare_dram_parameter("out", [S, D], f32, isOutput=True)

    Exp = mybir.ActivationFunctionType.Exp
    Ident = mybir.ActivationFunctionType.Identity
    Mult = mybir.AluOpType.mult

    with tile.TileContext(nc) as tc:
        with tc.tile_pool(name="const", bufs=1) as cpool:
            # ordered by consumption: hsT -> wv (phase2) -> wqk (3a) -> tables
            hsT = cpool.tile([128, 8, 1024], bf16, tag="hsT", name="hsT")
            nc.sync.dma_start(hsT[:], hsT_d.ap())
            wqk_sb = cpool.tile([128, 8, 2048], bf16, tag="wqk", name="wqk")
            nc.sync.dma_start(wqk_sb[:], wqk_d.ap())
            t2r_sb = cpool.tile([128, 2048], bf16, tag="t2r", name="t2r")
            nc.sync.dma_start(t2r_sb[:], t2r_d.ap())
            t2_sb = cpool.tile([128, 2048], bf16, tag="t2", name="t2")
            nc.sync.dma_start(t2_sb[:], t2_d.ap())
            wv_sb = cpool.tile([128, 8, 1024], bf16, tag="wv", name="wv")
            nc.sync.dma_start(wv_sb[:], wv_d.ap())
            bqk_sb = cpool.tile([128, 16], f32, tag="bqk", name="bqk")
            nc.sync.dma_start(bqk_sb[:], bqk_d.ap())
            bv_sb = cpool.tile([1, 1024], f32r, tag="bv", name="bv")
            nc.sync.dma_start(bv_sb[:], bv_d.ap())
            ones_sb = cpool.tile([1, 128], f32r, tag="ones", name="ones")
            nc.sync.dma_start(ones_sb[:], ones_d.ap())
            mhat_sb = cpool.tile([128, 8], f32, tag="mh", name="mh")
            nc.sync.dma_start(mhat_sb[:], mhat_d.ap())
            id8 = cpool.tile([128, 128], fp8, tag="id8", name="id8")
            nc.sync.dma_start(id8[:], id8_d.ap())
            idb = cpool.tile([65, 65], bf16, tag="idb", name="idb")
            nc.sync.dma_start(idb[:], idb_d.ap())

            # vh resident: [128, 8, 1040] bf16: [l_loc, tau, 65h+c]
            vh = cpool.tile([128, 8, 1040], bf16, tag="vh", name="vh")
            nc.gpsimd.memset(vh[:], 1.0)  # ones cols pre-set

            with tc.tile_pool(name="qk", bufs=2) as qkpool, \
                 tc.tile_pool(name="w8sb", bufs=8) as w8sb, \
                 tc.tile_pool(name="s1p", bufs=20) as s1pool, \
                 tc.tile_pool(name="s2p", bufs=20) as s2pool, \
                 tc.tile_pool(name="probs", bufs=4) as prpool, \
                 tc.tile_pool(name="csb", bufs=2) as csb, \
                 tc.tile_pool(name="recp", bufs=2) as recp, \
                 tc.tile_pool(name="outp", bufs=2) as outp, \
                 tc.tile_pool(name="bankp", bufs=6, space="PSUM") as bankp, \
                 tc.tile_pool(name="pcps", bufs=1, space="PSUM") as pcps:

                def emit_phase2():
                    for tau in range(NT):
                        for half in range(2):
                            hs_ = slice(512 * half, 512 * (half + 1))
                            psv = bankp.tile([128, 512], f32, tag="bank",
                                             name="vps")
                            nc.tensor.matmul(psv[:], ones_sb[:],
                                             bv_sb[:, hs_],
                                             start=True, stop=False,
                                             skip_group_check=True)
                            for it in range(8):
                                nc.tensor.matmul(
                                    psv[:],
                                    hsT[:, it, 128 * tau:128 * (tau + 1)],
                                    wv_sb[:, it, hs_],
                                    start=False, stop=(it == 7),
                                    skip_group_check=True)
                            dst = vh[:].__replace__(
                                ap=[[8 * 1040, 128], [65, 8], [1, 64]],
                                offset=1040 * tau + 65 * 8 * half)
                            srcap = psv[:].__replace__(
                                ap=[[512, 128], [64, 8], [1, 64]], offset=0)
                            nc.scalar.activation(dst, srcap, Ident)

                def emit_3a(P):
                    qT = qkpool.tile([128, 1024], bf16, tag="qT", name="qT")
                    kT = qkpool.tile([128, 1024], bf16, tag="kT", name="kT")
                    for dst_sb, ct in ((qT, P), (kT, 8 + P)):
                        for half in range(2):
                            hs_ = slice(512 * half, 512 * (half + 1))
                            ps = bankp.tile([128, 512], f32, tag="bank",
                                            name="qkps")
                            for it in range(8):
                                nc.tensor.matmul(
                                    ps[:],
                                    wqk_sb[:, it, 128 * ct:128 * (ct + 1)],
                                    hsT[:, it, hs_],
                                    start=(it == 0), stop=(it == 7),
                                    skip_group_check=True)
                            nc.scalar.activation(dst_sb[:, hs_], ps[:], Ident,
                                                 bias=bqk_sb[:, ct:ct + 1])
                    return qT, kT

                CHUNKS = ((0, 512), (512, 512), (1024, 128))

                def emit_windows(P, qT, kT):
                    s1 = [[None] * NT, [None] * NT]
                    s2 = [[None] * NT, [None] * NT]
                    for t in range(NT):
                        base = 896 - 128 * t
                        for hh in range(2):
                            rs = slice(64 * hh, 64 * (hh + 1))
                            w8q = w8sb.tile([128, WIN], fp8, tag="w8",
                                            name="w8q")
                            for ci, (c0, cw) in enumerate(CHUNKS):
                                wps = bankp.tile([128, 512], f32, tag="bank",
                                                 name="wps")
                                nc.tensor.matmul(
                                    wps[:, 0:cw],
                                    qT[rs, 128 * t:128 * (t + 1)],
                                    t2r_sb[rs, base + c0:base + c0 + cw],
                                    start=True, stop=True,
                                    skip_group_check=True)
                                if ci == 1:
                                    nc.vector.tensor_copy(
                                        w8q[:, c0:c0 + cw], wps[:, 0:cw])
                                else:
                                    nc.scalar.copy(
                                        w8q[:, c0:c0 + cw], wps[:, 0:cw])
                            blk = s1pool.tile([128, 1024], fp8, tag="s1",
                                              name="s1")
                            nc.sync.dma_start(blk[:], w8q[:].__replace__(
                                ap=[[1151, 128], [1, 1024]], offset=127))
                            s1[hh][t] = blk

                            w8k = w8sb.tile([128, WIN], fp8, tag="w8",
                                            name="w8k")
                            for ci, (c0, cw) in enumerate(CHUNKS):
                                wps = bankp.tile([128, 512], f32, tag="bank",
                                                 name="wpsk")
                                nc.tensor.matmul(
                                    wps[:, 0:cw],
                                    kT[rs, 128 * t:128 * (t + 1)],
                                    t2_sb[rs, base + c0:base + c0 + cw],
                                    start=True, stop=True,
                                    skip_group_check=True)
                                if ci == 1:
                                    nc.scalar.copy(
                                        w8k[:, c0:c0 + cw], wps[:, 0:cw])
                                else:
                                    nc.vector.tensor_copy(
                                        w8k[:, c0:c0 + cw], wps[:, 0:cw])
                            blk = s2pool.tile([128, 1024], fp8, tag="s2",
                                              name="s2")
                            nc.gpsimd.dma_start(blk[:], w8k[:].__replace__(
                                ap=[[1151, 128], [1, 1024]], offset=127))
                            s2[hh][t] = blk
                    return s1, s2

                def emit_heads(P, qT, kT, s1, s2):
                    outsb = outp.tile([128, 1024], f32, tag="osb", name="osb")
                    for hh in range(2):
                        h = 2 * P + hh
                        rs = slice(64 * hh, 64 * (hh + 1))
                        pc = pcps.tile([128, 1024], f32, tag="pc", name="ctx")
                        for t in range(NT):
                            pr = prpool.tile([128, 1024], bf16, tag="pr",
                                             name="pr")
                            for half in range(2):
                                sl = slice(512 * half, 512 * (half + 1))
                                sch = bankp.tile([128, 512], f32, tag="bank",
                                                 name="sc")
                                for Lj in range(4):
                                    L = 4 * half + Lj
                                    nc.tensor.matmul(
                                        sch[:, 128 * Lj:128 * (Lj + 1)],
                                        s1[hh][L][:, 128 * t:128 * (t + 1)],
                                        id8[:],
                                        start=(Lj == 0), stop=False,
                                        skip_group_check=True)
                                nc.tensor.matmul(sch[:],
                                                 kT[rs, 128 * t:128 * (t + 1)],
                                                 qT[rs, sl],
                                                 start=False, stop=False,
                                                 skip_group_check=True)
                                nc.tensor.matmul(sch[:], id8[:],
                                                 s2[hh][t][:, sl],
                                                 start=False, stop=True,
                                                 skip_group_check=True)
                                nc.scalar.activation(pr[:, sl], sch[:], Exp,
                                                     bias=mhat_sb[:, t:t + 1],
                                                     scale=0.125)
                            for half in range(2):
                                sl = slice(512 * half, 512 * (half + 1))
                                nc.tensor.matmul(
                                    pc[0:65, sl],
                                    vh[:].__replace__(
                                        ap=[[8 * 1040, 128], [1, 65]],
                                        offset=1040 * t + 65 * h),
                                    pr[:, sl],
                                    start=(t == 0), stop=(t == NT - 1),
                                    skip_group_check=True)

                        cs = csb.tile([65, 1024], bf16, tag="cs", name="cs")
                        nc.scalar.copy(cs[:], pc[0:65, :])
                        po = pcps.tile([128, 1024], f32, tag="pc", name="po")
                        for g in range(2):
                            for bk in range(4):
                                L = 4 * g + bk
                                nc.tensor.matmul(
                                    po[:, 512 * g + 65 * bk:
                                       512 * g + 65 * bk + 65],
                                    cs[:, 128 * L:128 * (L + 1)],
                                    idb[:],
                                    start=(bk == 0), stop=(bk == 3),
                                    skip_group_check=True)
                        rec = recp.tile([128, 8], f32, tag="rec", name="rec")
                        nc.vector.reciprocal(
                            rec[:].__replace__(ap=[[8, 128], [4, 2], [1, 4]],
                                               offset=0),
                            po[:].__replace__(
                                ap=[[1024, 128], [512, 2], [65, 4]],
                                offset=64))
                        nc.vector.tensor_tensor(
                            outsb[:].__replace__(
                                ap=[[1024, 128], [512, 2], [128, 4], [1, 64]],
                                offset=64 * hh),
                            po[:].__replace__(
                                ap=[[1024, 128], [512, 2], [65, 4], [1, 64]],
                                offset=0),
                            rec[:].__replace__(
                                ap=[[8, 128], [4, 2], [1, 4], [0, 64]],
                                offset=0),
                            op=Mult)
                    for L in range(NT):
                        nc.sync.dma_start(
                            out_d.ap()[128 * L:128 * (L + 1),
                                       128 * P:128 * (P + 1)],
                            outsb[:, 128 * L:128 * (L + 1)])

                # pair 0's projection + windows first (overlaps weight DMAs),
                # then phase 2, then the rest
                qT0, kT0 = emit_3a(0)
                s1_0, s2_0 = emit_windows(0, qT0, kT0)
                emit_phase2()
                emit_heads(0, qT0, kT0, s1_0, s2_0)
                for P in range(1, npair):
                    qT, kT = emit_3a(P)
                    s1, s2 = emit_windows(P, qT, kT)
                    emit_heads(P, qT, kT, s1, s2)

    nc.compile()
    return nc


_NC_CACHE = {}
_LAST = {"exec_time_ns": None}


def _get_program():
    if "nc" not in _NC_CACHE:
        _NC_CACHE["nc"] = build_program()
    return _NC_CACHE["nc"]


def get_last_exec_time_ns():
    return _LAST["exec_time_ns"]


def kernel(hidden_states, attention_mask, W_qkv, b_qkv, dist_emb):
    from concourse.bass_utils import run_bass_kernel_spmd

    hidden_states = np.asarray(hidden_states, dtype=np.float32)
    attention_mask = np.asarray(attention_mask, dtype=np.float32)
    W_qkv = np.asarray(W_qkv, dtype=np.float32)
    b_qkv = np.asarray(b_qkv, dtype=np.float32)
    dist_emb = np.asarray(dist_emb, dtype=np.float32)

    B = hidden_states.shape[0]
    nc = _get_program()
    in_maps = host_prep(hidden_states, attention_mask, W_qkv, b_qkv, dist_emb)
    trace = bool(os.environ.get("BASS_TRACE"))
    res = run_bass_kernel_spmd(nc, in_maps, list(range(B)), trace=trace)
    _LAST["exec_time_ns"] = res.exec_time_ns
    out = np.stack([res.results[i]["out"] for i in range(B)], axis=0)
    return out.astype(np.float32)
